# revision 8
# baseline (speedup 1.0000x reference)
"""Trainium2 Bass kernel for nn_DynamicsWithKnownReward.

Computes, for B=524288 rows:
  s_next = relu(relu([s,a] @ W1 + b1) @ W2 + b2) @ W3 + b3      (B, 3)
  r      = -(quad + gauss_obstacles + boundary)                 (B, 1)

Sharding: pure data parallel over 8 cores (65536 rows each). All layout
transforms happen host-side in numpy:
  - MLP inputs are fed transposed as xT (5, 65536) so the tiny weights are
    the PE-stationary operand and batch streams on the free dimension.
  - Reward inputs are fed as (4, 128, 512) tiles (batch spread over all 128
    partitions) so the elementwise engines run at full lane utilization.
Outputs come back as yT (3, 65536) = s_next^T and rout (128, 512) = r tiles;
the host transposes/reshapes them back.
"""

import math

import numpy as np

B = 524288
NCORES = 8
BPC = B // NCORES          # 65536 rows per core
CH = 1024                  # batch columns per pipeline chunk
NCH = BPC // CH            # 64 chunks
SC = 8192                  # columns per xT DMA super-chunk
NSC = BPC // SC            # 8 super-chunks
RF = BPC // 128            # 512: reward tile free dim

# Engine split of the two ReLU psum->sbuf passes: ACT takes the first
# F_ACT fraction of each chunk's columns, DVE the rest.
F_ACT = 0.56

VAR = 0.035
SIG = 0.03
OBS_Y = (0.0, 0.2, 0.4, 0.6, 0.8)        # obstacles 0-4 (x coord = 0)
# obstacle 5 sits at (-0.8, -0.8)
LNG = math.log(100.0 / (2.0 * math.pi * VAR))   # folded gaussian coefficient
GA = -0.5 / VAR                                  # exp scale for obstacles
LNB = math.log(10.0 / (SIG * math.sqrt(2.0 * math.pi)))  # boundary coef
BA = -0.5 / (SIG * SIG)                          # exp scale for boundaries

_CACHED_NC = None


def _build_nc():
    import concourse.bacc as bacc
    import concourse.mybir as mybir
    from concourse.tile import TileContext

    F32R = mybir.dt.float32r
    F32 = mybir.dt.float32
    AF = mybir.ActivationFunctionType
    ALU = mybir.AluOpType

    nc = bacc.Bacc(None, target_bir_lowering=False)

    xT = nc.declare_dram_parameter("xT", [5, BPC], F32R, isOutput=False)
    rin = nc.declare_dram_parameter("rin", [4, 128, RF], F32, isOutput=False)
    W1 = nc.declare_dram_parameter("W1", [5, 128], F32R, isOutput=False)
    W2 = nc.declare_dram_parameter("W2", [128, 128], F32R, isOutput=False)
    W3P = nc.declare_dram_parameter("W3P", [4, 128, 128], F32R, isOutput=False)
    b1 = nc.declare_dram_parameter("b1", [128, 1], F32, isOutput=False)
    b2 = nc.declare_dram_parameter("b2", [128, 1], F32, isOutput=False)
    b3t = nc.declare_dram_parameter("b3t", [128, 1], F32, isOutput=False)
    yT = nc.declare_dram_parameter("yT", [3, BPC], F32, isOutput=True)
    rout = nc.declare_dram_parameter("rout", [128, RF], F32, isOutput=True)

    CA = int(CH * F_ACT)  # ACT columns per relu chunk

    with TileContext(nc) as tc:
        with tc.tile_pool(name="const", bufs=1) as cp, \
             tc.tile_pool(name="xin", bufs=2) as xp, \
             tc.tile_pool(name="h", bufs=3) as hp, \
             tc.tile_pool(name="stg", bufs=2) as sp, \
             tc.tile_pool(name="rw", bufs=1) as rp, \
             tc.tile_pool(name="ps", bufs=3, space="PSUM") as pp, \
             tc.tile_pool(name="psy", bufs=2, space="PSUM") as pyp:

            w1t = cp.tile([5, 128], F32R)
            nc.sync.dma_start(out=w1t[:], in_=W1[:])
            w2t = cp.tile([128, 128], F32R)
            nc.sync.dma_start(out=w2t[:], in_=W2[:])
            w3s = []
            for p in range(4):
                w3p = cp.tile([128, 128], F32R, tag=f"w3_{p}", name=f"w3_{p}")
                nc.sync.dma_start(out=w3p[:], in_=W3P[p])
                w3s.append(w3p)
            b1t = cp.tile([128, 1], F32)
            nc.sync.dma_start(out=b1t[:], in_=b1[:])
            b2t = cp.tile([128, 1], F32)
            nc.sync.dma_start(out=b2t[:], in_=b2[:])
            b3tt = cp.tile([128, 1], F32)
            nc.sync.dma_start(out=b3tt[:], in_=b3t[:])
            lng_t = cp.tile([128, 1], F32)
            nc.vector.memset(lng_t[:], LNG)
            lnb_t = cp.tile([128, 1], F32)
            nc.vector.memset(lnb_t[:], LNB)

            # ---------------- reward (independent of the MLP) ----------------
            rs0 = rp.tile([128, RF], F32, tag="rs0")
            nc.sync.dma_start(out=rs0[:], in_=rin[0])
            rs1 = rp.tile([128, RF], F32, tag="rs1")
            nc.sync.dma_start(out=rs1[:], in_=rin[1])
            ra0 = rp.tile([128, RF], F32, tag="ra0")
            nc.sync.dma_start(out=ra0[:], in_=rin[2])
            ra1 = rp.tile([128, RF], F32, tag="ra1")
            nc.sync.dma_start(out=ra1[:], in_=rin[3])

            def rt(tag):
                return rp.tile([128, RF], F32, tag="rtmp", name=tag,
                               bufs=16)

            # gaussian obstacles 0-4: exp(-(s1-y)^2/2V) summed, times
            # exp(-s0^2/2V + LNG)
            sq0 = rt("sq0")
            nc.vector.tensor_mul(sq0[:], rs0[:], rs0[:])
            ex0 = rt("ex0")
            nc.scalar.activation(ex0[:], sq0[:], AF.Exp, bias=lng_t[:], scale=GA)
            eys = []
            for i, y in enumerate(OBS_Y):
                if y == 0.0:
                    d = rs1
                else:
                    d = rt(f"d{i}")
                    nc.gpsimd.tensor_scalar_add(d[:], rs1[:], -y)
                q = rt(f"q{i}")
                nc.vector.tensor_mul(q[:], d[:], d[:])
                e = rt(f"e{i}")
                nc.scalar.activation(e[:], q[:], AF.Exp, bias=0.0, scale=GA)
                eys.append(e)
            ey01 = rt("ey01")
            nc.gpsimd.tensor_add(ey01[:], eys[0][:], eys[1][:])
            ey23 = rt("ey23")
            nc.gpsimd.tensor_add(ey23[:], eys[2][:], eys[3][:])
            ey = rt("ey")
            nc.vector.tensor_add(ey[:], ey01[:], ey23[:])
            ey4 = rt("ey4")
            nc.vector.tensor_add(ey4[:], ey[:], eys[4][:])
            g04 = rt("g04")
            nc.vector.tensor_mul(g04[:], ex0[:], ey4[:])

            # obstacle 5 at (-0.8, -0.8): exp(-((s0+.8)^2+(s1+.8)^2)/2V + LNG)
            u5a = rt("u5a")
            nc.gpsimd.tensor_scalar_add(u5a[:], rs0[:], 0.8)
            u5b = rt("u5b")
            nc.gpsimd.tensor_scalar_add(u5b[:], rs1[:], 0.8)
            v5a = rt("v5a")
            nc.vector.tensor_mul(v5a[:], u5a[:], u5a[:])
            v5 = rt("v5")
            nc.vector.scalar_tensor_tensor(
                v5[:], u5b[:], 1.0, u5b[:], ALU.mult, ALU.mult)
            t5 = rt("t5")
            nc.vector.tensor_add(t5[:], v5a[:], v5[:])
            g5 = rt("g5")
            nc.scalar.activation(g5[:], t5[:], AF.Exp, bias=lng_t[:], scale=GA)
            gauss = rt("gauss")
            nc.vector.tensor_add(gauss[:], g04[:], g5[:])

            # boundary gaussians at x=+-1.5, y=+-1.0
            bes = []
            for i, (src, c) in enumerate(
                    ((rs0, 1.5), (rs0, -1.5), (rs1, -1.0), (rs1, 1.0))):
                bd = rt(f"bd{i}")
                nc.gpsimd.tensor_scalar_add(bd[:], src[:], c)
                bq = rt(f"bq{i}")
                nc.vector.tensor_mul(bq[:], bd[:], bd[:])
                be = rt(f"be{i}")
                nc.scalar.activation(be[:], bq[:], AF.Exp, bias=lnb_t[:], scale=BA)
                bes.append(be)
            b01 = rt("b01")
            nc.gpsimd.tensor_add(b01[:], bes[0][:], bes[1][:])
            b23 = rt("b23")
            nc.gpsimd.tensor_add(b23[:], bes[2][:], bes[3][:])
            bsum = rt("bsum")
            nc.vector.tensor_add(bsum[:], b01[:], b23[:])

            # quadratic cost 30*((s0-a0)^2 + (s1-a1)^2)
            d0 = rt("d0")
            nc.vector.tensor_sub(d0[:], rs0[:], ra0[:])
            d1 = rt("d1")
            nc.gpsimd.tensor_sub(d1[:], rs1[:], ra1[:])
            q0 = rt("q0")
            nc.vector.tensor_mul(q0[:], d0[:], d0[:])
            qd = rt("qd")
            nc.vector.scalar_tensor_tensor(
                qd[:], d1[:], 1.0, d1[:], ALU.mult, ALU.mult)
            qsum = rt("qsum")
            nc.vector.tensor_add(qsum[:], q0[:], qd[:])

            tot = rt("tot")
            nc.vector.tensor_add(tot[:], gauss[:], bsum[:])
            rres = rt("rres")
            # r = (q * -30) - tot
            nc.vector.scalar_tensor_tensor(
                rres[:], qsum[:], -30.0, tot[:], ALU.mult, ALU.subtract)
            nc.sync.dma_start(out=rout[:], in_=rres[:])

            # ---------------- MLP over 64 chunks of 1024 columns -------------
            psY = None
            stg = None
            for s in range(NSC):
                xt = xp.tile([5, SC], F32R, tag="xt")
                nc.sync.dma_start(out=xt[:], in_=xT[:, s * SC:(s + 1) * SC])
                stg = sp.tile([128, 4 * 512], F32, tag="stg")
                for jj in range(SC // CH):
                    j = s * (SC // CH) + jj
                    # layer 1
                    ps1 = pp.tile([128, CH], F32, tag="ps")
                    for k in range(CH // 512):
                        nc.tensor.matmul(
                            ps1[:, k * 512:(k + 1) * 512], w1t[:],
                            xt[:, jj * CH + k * 512: jj * CH + (k + 1) * 512],
                            start=True, stop=True)
                    h1 = hp.tile([128, CH], F32R, tag="h1")
                    nc.scalar.activation(h1[:, :CA], ps1[:, :CA], AF.Relu,
                                         bias=b1t[:])
                    nc.vector.tensor_scalar(h1[:, CA:], ps1[:, CA:], b1t[:],
                                            0.0, ALU.add, ALU.max)
                    # layer 2
                    ps2 = pp.tile([128, CH], F32, tag="ps")
                    for k in range(CH // 512):
                        nc.tensor.matmul(
                            ps2[:, k * 512:(k + 1) * 512], w2t[:],
                            h1[:, k * 512:(k + 1) * 512],
                            start=True, stop=True)
                    h2 = hp.tile([128, CH], F32R, tag="h2")
                    nc.scalar.activation(h2[:, :CA], ps2[:, :CA], AF.Relu,
                                         bias=b2t[:])
                    nc.vector.tensor_scalar(h2[:, CA:], ps2[:, CA:], b2t[:],
                                            0.0, ALU.add, ALU.max)
                    # layer 3, col-packed 4 subchunks per psY tile
                    for k in range(CH // 512):
                        g = j * (CH // 512) + k          # global subchunk
                        pos = g % 4
                        if pos == 0:
                            psY = pyp.tile([128, 512], F32, tag="psy")
                        # four shifted zero-padded W3 copies accumulate into
                        # one psum tile, placing each subchunk's 3 output
                        # rows at partition stripe 32*pos
                        nc.tensor.matmul(
                            psY[:, :], w3s[pos][:],
                            h2[:, k * 512:(k + 1) * 512],
                            start=(pos == 0), stop=(pos == 3))
                        if pos == 3:
                            gg = (g // 4) % 4            # group within stage
                            if gg % 2 == 0:
                                nc.scalar.activation(
                                    stg[:, gg * 512:(gg + 1) * 512], psY[:],
                                    AF.Identity, bias=b3tt[:])
                            else:
                                nc.vector.tensor_scalar_add(
                                    stg[:, gg * 512:(gg + 1) * 512], psY[:],
                                    b3tt[:])
                # store the 4 stripes of this super-chunk
                ysl = yT[:, s * SC:(s + 1) * SC].rearrange(
                    "r (g i f) -> r g i f", i=4, f=512)
                for i in range(4):
                    src = stg[32 * i:32 * i + 3, :].rearrange(
                        "r (g f) -> r g f", f=512)
                    nc.sync.dma_start(out=ysl[:, :, i, :], in_=src)

    nc.compile()
    return nc


def _get_nc():
    global _CACHED_NC
    if _CACHED_NC is None:
        _CACHED_NC = _build_nc()
    return _CACHED_NC


def make_in_maps(s, a, W1, b1, W2, b2, W3, b3):
    f32 = np.float32
    s = np.asarray(s, f32)
    a = np.asarray(a, f32)
    W1 = np.ascontiguousarray(np.asarray(W1, f32))
    W2 = np.ascontiguousarray(np.asarray(W2, f32))
    W3 = np.asarray(W3, f32)
    W3P = np.zeros((4, 128, 128), f32)
    for p in range(4):
        W3P[p, :, 32 * p:32 * p + 3] = W3
    b1c = np.asarray(b1, f32).reshape(128, 1)
    b2c = np.asarray(b2, f32).reshape(128, 1)
    b3c = np.asarray(b3, f32).reshape(3)
    b3t = np.zeros((128, 1), f32)
    for i in range(4):
        b3t[32 * i:32 * i + 3, 0] = b3c
    in_maps = []
    for c in range(NCORES):
        sl = slice(c * BPC, (c + 1) * BPC)
        sc = s[sl]
        ac = a[sl]
        xTc = np.ascontiguousarray(np.concatenate([sc.T, ac.T], axis=0))
        rinc = np.ascontiguousarray(np.stack([
            sc[:, 0].reshape(128, RF), sc[:, 1].reshape(128, RF),
            ac[:, 0].reshape(128, RF), ac[:, 1].reshape(128, RF)]))
        in_maps.append({
            "xT": xTc, "rin": rinc, "W1": W1, "W2": W2, "W3P": W3P,
            "b1": b1c, "b2": b2c, "b3t": b3t,
        })
    return in_maps


def assemble_outputs(results):
    f32 = np.float32
    s_next = np.empty((B, 3), f32)
    r = np.empty((B, 1), f32)
    for c in range(NCORES):
        sl = slice(c * BPC, (c + 1) * BPC)
        s_next[sl] = np.asarray(results[c]["yT"]).T
        r[sl, 0] = np.asarray(results[c]["rout"]).reshape(BPC)
    return s_next, r


def kernel(s, a, W1, b1, W2, b2, W3, b3):
    from concourse.bass_utils import run_bass_kernel_spmd

    nc = _get_nc()
    in_maps = make_in_maps(s, a, W1, b1, W2, b2, W3, b3)
    res = run_bass_kernel_spmd(nc, in_maps, list(range(NCORES)))
    return assemble_outputs(res.results)
